# revision 1
# baseline (speedup 1.0000x reference)
"""CrossScaleAttention Trainium2 kernel.

Full (unsharded) contract: kernel(query, key, value) with shapes
  query/key/value: (4, 4096, 256) float32  ->  out (4, 4096, 256) float32

reference math:
  q = l2norm(query); k = l2norm(key)
  out = softmax((q @ k^T) * 32**-0.5) @ value

Sharding: 8 cores; core c computes batch c//2, query rows (c%2)*2048..+2048,
with that batch's full K/V resident per core (no collectives needed).

Per-core algorithm (all PE matmuls contract over the partition dim):
  - K is PE-transposed RAW into K^T [d=128x2, keys]; its row norm (and the
    global softmax scale) ride the exp as a per-partition scale, so K^T
    production depends only on the DMA. Q is scaled to unit rows before its
    transpose (a per-query scale cannot ride the exp, whose scale AP is
    per-partition = per-key in the S^T layout).
  - row norms: sum-of-squares via one DVE mul+reduce pair per tile group;
    rsqrt on DVE via the 0x5f3759df bit trick + 2 Newton steps. ACT runs a
    single Exp table the whole kernel -- zero table switches.
  - S^T chunks [128 keys, 512 queries]: K^T/Q^T are stored fp8e4m3 and one
    DoubleRow matmul per chunk contracts all 256 d at 2 MACs/cell/cycle
    (the [128, 2, n] d-major layout is exactly DoubleRow's interleave).
    Scores are cosine similarities so fp8 inputs cost ~8e-4 output error.
  - exp(scale_k * s) via ACT straight PSUM->SBUF producing P^T in f32r.
    No max-subtraction needed: |logit| <= 0.177, exp cannot overflow.
  - AV (f32r, full-rate fp32 PE mode): out_psum[128 q, 258] +=
    P^T_chunk.T @ [V | 1 1]; the ones columns accumulate the softmax
    denominator inside the same accumulation chain (padded to 258 because
    f32r matmuls need an even moving-dim).
  - epilogue: out = out_psum[:, :256] * (1 / out_psum[:, 256]).
  - inputs arrive via a few large multi-tile DMAs (one InstDMACopy spreads
    across all 16 SDMA slots); the prologue tail (norms, V staging copies,
    transposes) is interleaved into block 0's emission so every in-order
    engine stream matches data arrival. Early PSUM->SBUF copies run on the
    then-idle ACT engine; steady state is PE/ACT co-limited at ~99% PE busy.

Measured on 8 axon TRN2 cores: ~149 us HW exec, absmax relative error
~7.9e-4 vs the fp32 reference (l2 rel ~4.5e-4).
"""

import sys

if "/opt/trn_rl_repo" not in sys.path:
    sys.path.insert(0, "/opt/trn_rl_repo")

import numpy as np

import concourse.bass as bass
import concourse.mybir as mybir
import concourse.tile as tile
from concourse import bacc
from concourse.bass_utils import run_bass_kernel_spmd
from concourse.masks import make_identity

F32 = mybir.dt.float32
F32R = mybir.dt.float32r
FP8 = mybir.dt.float8e4
I32 = mybir.dt.int32

B, NQ_FULL, NK, D = 4, 4096, 4096, 256
N_CORES = 8
NQ = NQ_FULL * B // N_CORES  # 2048 queries per core
P = 128
DC = D // P          # 2 d-chunks
KC = NK // P         # 32 key chunks
QTI = NQ // P        # 16 q tiles
QB = 512             # queries per block
NB = NQ // QB        # 4 blocks
QT = QB // P         # 4 q-subtiles per block
VW = D + 2           # V columns padded with two 1.0 columns (even moving-dim)
SC = 2               # key chunks per super-chunk (shared exp)
NT = KC + QTI        # 48 row tiles total
SCALE = float(D // 8) ** -0.5  # head_dim**-0.5 = 32**-0.5
RSQRT_MAGIC = 0x5F3759DF

Exp = mybir.ActivationFunctionType.Exp

# natall/ssall/rinv_all positions: q0-3 -> 0..3, k0-31 -> 4..35, q4-15 -> 36..47
QPOS0, KPOS, QPOS1 = 0, 4, 36


def _build_program():
    nc = bacc.Bacc(
        "TRN2",
        target_bir_lowering=False,
        debug=False,
        enable_asserts=False,
        num_devices=N_CORES,
    )
    q_d = nc.dram_tensor("q", (NQ, D), F32, kind="ExternalInput").ap()
    k_d = nc.dram_tensor("k", (NK, D), F32, kind="ExternalInput").ap()
    v_d = nc.dram_tensor("v", (NK, D), F32, kind="ExternalInput").ap()
    o_d = nc.dram_tensor("o", (NQ, D), F32, kind="ExternalOutput").ap()

    k_re = k_d.rearrange("(i p) d -> p i d", p=P)  # [128, 32, 256]
    q_re = q_d.rearrange("(i p) d -> p i d", p=P)  # [128, 16, 256]
    v_re = v_d.rearrange("(i p) d -> p i d", p=P)  # [128, 32, 256]

    with tile.TileContext(nc) as tc:
        with (
            tc.tile_pool(name="const", bufs=1) as const_pool,
            tc.tile_pool(name="persist", bufs=1) as persist,
            tc.tile_pool(name="stage", bufs=2) as stage,
            tc.tile_pool(name="loads", bufs=4) as loads,
            tc.tile_pool(name="small", bufs=8) as small,
            tc.tile_pool(name="pt", bufs=4) as pt_pool,
            tc.tile_pool(name="outs", bufs=3) as out_pool,
            tc.tile_pool(name="ps", bufs=4, space="PSUM") as ps_pool,
            tc.tile_pool(name="avps", bufs=1, space="PSUM") as av_pool,
        ):
            ident = const_pool.tile([P, P], F32)
            make_identity(nc, ident)
            ones = const_pool.tile([P, 1], F32)
            nc.vector.memset(ones, 1.0)
            magic = const_pool.tile([P, 1], I32)
            nc.vector.memset(magic, RSQRT_MAGIC)

            # persistent operands
            kt = persist.tile([P, DC, NK], FP8)     # K^T: [d, keys] (RAW rows)
            qt = persist.tile([P, DC, NQ], FP8)     # Q^T: [d, queries] unit-norm
            va = persist.tile([P, KC, VW], F32R)    # [keys, d | ones ones]
            natall = persist.tile([P, NT, D], F32)  # raw rows
            ssall = persist.tile([P, NT], F32)      # row sum-of-squares
            rinv_all = persist.tile([P, NT], F32)   # (pre)scale / ||row||

            nc.vector.tensor_copy(
                va[:, :, D:VW], ones[:, :, None].to_broadcast((P, KC, 2))
            )

            # ---- input DMAs: a few multi-tile transfers, block-0 deps first
            nc.sync.dma_start(natall[:, 0:4, :], q_re[:, 0:4, :])      # q0-3
            nc.sync.dma_start(natall[:, 4:6, :], k_re[:, 0:2, :])      # k0-1
            nc.sync.dma_start(natall[:, 6:12, :], k_re[:, 2:8, :])     # k2-7
            vstg = []
            for g, (v0, v1) in enumerate(((0, 8), (8, 16), (16, 24), (24, KC))):
                vs = stage.tile([P, 8, D], F32, tag="vl", name=f"vs{g}")
                vstg.append((vs, v0, v1))
            nc.sync.dma_start(vstg[0][0], v_re[:, 0:8, :])
            nc.sync.dma_start(vstg[1][0], v_re[:, 8:16, :])
            nc.sync.dma_start(natall[:, 12:20, :], k_re[:, 8:16, :])   # k8-15
            nc.sync.dma_start(vstg[2][0], v_re[:, 16:24, :])
            nc.sync.dma_start(natall[:, 20:36, :], k_re[:, 16:KC, :])  # k16-31
            nc.sync.dma_start(vstg[3][0], v_re[:, 24:KC, :])
            nc.sync.dma_start(natall[:, 36:48, :], q_re[:, 4:QTI, :])  # q4-15

            # V copies (f32 -> f32r round): first chunk on idle ACT; the rest
            # in small per-2-tile pieces the DVE scheduler can interleave,
            # emitted below at their DMA-arrival points.
            nc.scalar.copy(va[:, 0:8, :D], vstg[0][0])

            def v_copies(g):
                vs, v0, v1 = vstg[g]
                for j in range(0, v1 - v0, 2):
                    nc.vector.tensor_copy(
                        va[:, v0 + j : v0 + j + 2, :D], vs[:, j : j + 2, :]
                    )

            # ---- row norms (all DVE; no ACT tables involved) ----
            def norms(lo, hi, q_scale):
                n = hi - lo
                sq = stage.tile([P, n, D], F32, tag="sqg", name=f"sqg{lo}")
                nat = natall[:, lo:hi, :]
                nc.vector.tensor_mul(sq, nat, nat)
                ss = ssall[:, lo:hi]
                nc.vector.tensor_reduce(
                    ss, sq, axis=mybir.AxisListType.X, op=mybir.AluOpType.add
                )
                # rsqrt: bit-trick seed + 2 Newton iterations (err ~5e-6)
                y = rinv_all[:, lo:hi]
                yi = y.bitcast(I32)
                nc.vector.tensor_scalar(
                    yi, ss.bitcast(I32), 1, None,
                    op0=mybir.AluOpType.logical_shift_right,
                )
                nc.vector.tensor_tensor(
                    yi, magic.to_broadcast((P, n)), yi, mybir.AluOpType.subtract
                )
                t = small.tile([P, n], F32, tag="nt", name=f"nt{lo}")
                for _ in range(2):
                    nc.vector.tensor_mul(t, y, y)
                    nc.vector.tensor_mul(t, t, ss)
                    nc.vector.tensor_scalar(
                        t, t, -0.5, 1.5,
                        op0=mybir.AluOpType.mult, op1=mybir.AluOpType.add,
                    )
                    nc.vector.tensor_mul(y, y, t)
                if q_scale:
                    nc.vector.tensor_scalar_mul(y, y, SCALE)

            def finish(pos, kind, idx, copy_eng):
                """PE-transpose row-tile `pos` into kt/qt column idx."""
                if kind == "q":
                    src = loads.tile([P, D], F32, tag="xn", name=f"xn{pos}")
                    nc.vector.tensor_scalar_mul(
                        src, natall[:, pos, :], rinv_all[:, pos : pos + 1]
                    )
                    dst = qt
                else:
                    src = natall[:, pos, :]
                    dst = kt
                tps = ps_pool.tile([P, QB], F32, tag="st", name=f"tp{pos}")
                for dc in range(DC):
                    nc.tensor.transpose(
                        tps[:, dc * P : (dc + 1) * P],
                        src[:, dc * P : (dc + 1) * P],
                        ident,
                    )
                csrc = tps[:, :D].rearrange("p (c n) -> p c n", c=DC)
                cdst = dst[:, :, idx * P : (idx + 1) * P]
                if copy_eng == "scalar":
                    nc.scalar.copy(cdst, csrc)
                else:
                    nc.vector.tensor_copy(cdst, csrc)

            # part 1: everything block 0's first chunks need
            norms(0, 4, False)                      # q0-3
            for i in range(4):
                finish(QPOS0 + i, "q", i, "scalar")
            norms(4, 12, True)                      # k0-7
            for i in range(8):
                finish(KPOS + i, "k", i, "scalar")
            norms(12, 20, True)                     # k8-15
            v_copies(1)                             # v8-15

            # the rest of the prologue is interleaved into block 0's chunk
            # emission so every in-order engine stream matches data arrival
            after_chunk = {
                1: [lambda: [finish(KPOS + i, "k", i, "scalar") for i in range(8, 12)]],
                3: [lambda: [finish(KPOS + i, "k", i, "scalar") for i in range(12, 16)]],
                7: [lambda: v_copies(2), lambda: norms(20, 28, True)],
                9: [lambda: [finish(KPOS + i, "k", i, "scalar") for i in range(16, 20)]],
                11: [lambda: [finish(KPOS + i, "k", i, "scalar") for i in range(20, 24)]],
                13: [lambda: norms(28, 36, True), lambda: v_copies(3)],
                15: [lambda: [finish(KPOS + i, "k", i, "scalar") for i in range(24, 28)]],
                17: [lambda: [finish(KPOS + i, "k", i, "scalar") for i in range(28, KC)]],
                19: [lambda: norms(36, 48, False)],
                21: [lambda: [finish(QPOS1 + i - 4, "q", i, "scalar") for i in range(4, 8)]],
                23: [lambda: [finish(QPOS1 + i - 4, "q", i, "scalar") for i in range(8, 12)]],
                25: [lambda: [finish(QPOS1 + i - 4, "q", i, "scalar") for i in range(12, QTI)]],
            }

            # ---- main loop ----
            for blk in range(NB):
                avs = [
                    av_pool.tile([P, VW], F32, tag=f"av{t}", name=f"av{t}_{blk}")
                    for t in range(QT)
                ]
                for kk in range(KC):
                    st = ps_pool.tile([P, QB], F32, tag="st", name=f"st{blk}_{kk}")
                    nc.tensor.matmul(
                        st,
                        lhsT=kt[:, :, kk * P : (kk + 1) * P],
                        rhs=qt[:, :, blk * QB : (blk + 1) * QB],
                        start=True,
                        stop=True,
                        perf_mode=mybir.MatmulPerfMode.DoubleRow,
                    )
                    pt = pt_pool.tile([P, QB], F32R, tag="pt", name=f"pt{blk}_{kk}")
                    nc.scalar.activation(
                        pt, st, Exp, scale=rinv_all[:, KPOS + kk : KPOS + kk + 1]
                    )
                    for t in range(QT):
                        nc.tensor.matmul(
                            avs[t],
                            lhsT=pt[:, t * P : (t + 1) * P],
                            rhs=va[:, kk, :],
                            start=(kk == 0),
                            stop=(kk == KC - 1),
                        )
                    if blk == 0:
                        for thunk in after_chunk.get(kk, ()):
                            thunk()
                for t in range(QT):
                    rec = small.tile([P, 1], F32, tag="rec")
                    nc.vector.reciprocal(rec, avs[t][:, D : D + 1])
                    ot = out_pool.tile([P, D], F32, tag="ot")
                    nc.vector.tensor_scalar_mul(ot, avs[t][:, :D], rec)
                    row = blk * QB + t * P
                    nc.sync.dma_start(o_d[row : row + P, :], ot)

    nc.compile()
    return nc


_CACHED = {}


def _get_program():
    if "nc" not in _CACHED:
        _CACHED["nc"] = _build_program()
    return _CACHED["nc"]


def _get_runner():
    """Cached jitted shard_map executor (run_bass_via_pjrt rebuilds its jit
    wrapper on every call; caching it saves ~1-2s of retrace per invocation)."""
    if "runner" in _CACHED:
        return _CACHED["runner"]
    import jax
    from jax.sharding import Mesh, PartitionSpec
    from jax.experimental.shard_map import shard_map
    from concourse import bass2jax
    import concourse.mybir as _mb

    nc = _get_program()
    bass2jax.install_neuronx_cc_hook()

    partition_name = nc.partition_id_tensor.name if nc.partition_id_tensor else None
    in_names, out_names, out_avals, zero_outs = [], [], [], []
    for alloc in nc.m.functions[0].allocations:
        if not isinstance(alloc, _mb.MemoryLocationSet):
            continue
        name = alloc.memorylocations[0].name
        if alloc.kind == "ExternalInput":
            if name != partition_name:
                in_names.append(name)
        elif alloc.kind == "ExternalOutput":
            shape = tuple(alloc.tensor_shape)
            npdt = _mb.dt.np(alloc.dtype)
            out_names.append(name)
            out_avals.append(jax.core.ShapedArray(shape, npdt))
            zero_outs.append(np.zeros(shape, npdt))
    n_params = len(in_names)
    n_outs = len(out_names)
    all_names = in_names + out_names
    if partition_name is not None:
        all_names = all_names + [partition_name]
    donate = tuple(range(n_params, n_params + n_outs))

    def _body(*args):
        operands = list(args)
        if partition_name is not None:
            operands.append(bass2jax.partition_id_tensor())
        outs = bass2jax._bass_exec_p.bind(
            *operands,
            out_avals=tuple(out_avals),
            in_names=tuple(all_names),
            out_names=tuple(out_names),
            lowering_input_output_aliases=(),
            sim_require_finite=True,
            sim_require_nnan=True,
            nc=nc,
        )
        return tuple(outs)

    devices = jax.devices()[:N_CORES]
    mesh = Mesh(np.asarray(devices), ("core",))
    sharded = jax.jit(
        shard_map(
            _body,
            mesh=mesh,
            in_specs=(PartitionSpec("core"),) * (n_params + n_outs),
            out_specs=(PartitionSpec("core"),) * n_outs,
            check_rep=False,
        ),
        donate_argnums=donate,
        keep_unused=True,
    )

    def run(in_maps):
        concat_in = [
            np.concatenate([m[name] for m in in_maps], axis=0) for name in in_names
        ]
        concat_zeros = [
            np.zeros((N_CORES * z.shape[0], *z.shape[1:]), z.dtype) for z in zero_outs
        ]
        out_arrs = sharded(*concat_in, *concat_zeros)
        return [
            {
                name: np.asarray(out_arrs[i]).reshape(N_CORES, *out_avals[i].shape)[c]
                for i, name in enumerate(out_names)
            }
            for c in range(N_CORES)
        ]

    _CACHED["runner"] = run
    return run


def _make_in_maps(query, key, value):
    in_maps = []
    for c in range(N_CORES):
        b = c // (N_CORES // B)
        qs = (c % (N_CORES // B)) * NQ
        in_maps.append(
            {
                "q": np.ascontiguousarray(query[b, qs : qs + NQ], dtype=np.float32),
                "k": np.ascontiguousarray(key[b], dtype=np.float32),
                "v": np.ascontiguousarray(value[b], dtype=np.float32),
            }
        )
    return in_maps


def _gather(results):
    out = np.empty((B, NQ_FULL, D), dtype=np.float32)
    for c in range(N_CORES):
        b = c // (N_CORES // B)
        qs = (c % (N_CORES // B)) * NQ
        out[b, qs : qs + NQ] = results[c]["o"]
    return out


def run_sharded(query, key, value, trace=False):
    """Returns (out, BassKernelResults). trace=True goes through the
    profiling path; the fast path uses the cached jitted executor."""
    in_maps = _make_in_maps(query, key, value)
    if trace:
        nc = _get_program()
        res = run_bass_kernel_spmd(
            nc, in_maps, core_ids=list(range(N_CORES)), trace=True
        )
        return _gather(res.results), res
    run = _get_runner()
    return _gather(run(in_maps)), None


def kernel(query, key, value):
    query = np.asarray(query)
    key = np.asarray(key)
    value = np.asarray(value)
    try:
        out, _ = run_sharded(query, key, value)
    except Exception:
        # fall back to the framework executor if the cached-runner fast
        # path hits an incompatibility
        nc = _get_program()
        in_maps = _make_in_maps(query, key, value)
        res = run_bass_kernel_spmd(nc, in_maps, core_ids=list(range(N_CORES)))
        out = _gather(res.results)
    return out

